# revision 2
# baseline (speedup 1.0000x reference)
"""AdaPT quantized linear (int8-exact via fp16 matmul) on 8 TRN2 NeuronCores.

Reference computes:
    qx = clip(round(x * 127/amax_x), -127, 127)        [N, K] int8
    qw = clip(round(w * 127/amax_w), -127, 127)        [M, K] int8
    out = (qx @ qw.T) / ((127/amax_x)*(127/amax_w)) + bias

Strategy: data-parallel over the 8192-token dim (1024 tokens/core), full
weight on every core, no collectives.  All int8 values are exactly
representable in fp16, the PE multiplies fp16 exactly (products < 2^14)
and accumulates in fp32 (partial sums << 2^24), so the fp16 matmul
reproduces the int8 systolic GEMM bit-exactly.

Rounding trick: fp16 has unit spacing on [1024, 2048), so converting
(x*scale + 1536) from fp32 to fp16 rounds the fractional part to the
nearest integer with ties-to-even -- exactly jnp.round.  The quant chain
is then: fp32->fp16 affine convert (ScalarE Copy, immediate bias -- no
const-AP memsets or engine barrier needed), fp16 min/max clip to
[1536-127, 1536+127] (DVE dual-op at 16-bit rate), fp16 subtract 1536.

Per-core device kernel (weight-stationary, m-tile outer loop):
  - quantize x.T shard once into resident SBUF fp16 [128, 2, 1024] pairs
  - per m-tile (128 rows of W): DMA w.T block in 4 chunks, quantize, 64
    accumulating matmuls (32 k-tiles x 2 token chunks of 512), dequant+
    bias on ScalarE straight out of PSUM, DMA out.
  - prologue interleaves x-quant chunks with the first 4 m-tiles at fine
    (sub-chunk) granularity so the PE starts real matmuls ~10us in; a
    short junk-matmul burst covers the PE pstate ramp before that.

Output is produced transposed ([M, tokens/core] per core) so the bias can
ride the ScalarE per-partition bias port; host transposes back.
"""

import sys

import numpy as np

sys.path.insert(0, "/opt/trn_rl_repo")

N, K, M = 8192, 4096, 4096
N_CORES = 8
TPC = N // N_CORES  # tokens per core
P = 128
KT = K // P   # 32 k-tiles
MT = M // P   # 32 m-tiles
TF = 512      # matmul moving free dim (one PSUM bank of fp32)
NTF = TPC // TF
XCH = 4       # resident xq is split into chunks for finer scheduling deps
KPC = KT // XCH
MAGIC = 1536.0   # fp16 unit-spacing window base: round via fp32->fp16 convert
MAXV = 127.0
HI = MAGIC + MAXV
LO = MAGIC - MAXV
JUNK = 12     # PE pstate-ramp warmup matmuls


def build(s_x: float, s_w: float, inv_s: float):
    import concourse.mybir as mybir
    import concourse.tile as tile
    from concourse import bacc

    dt = mybir.dt
    AF = mybir.ActivationFunctionType
    OP = mybir.AluOpType

    nc = bacc.Bacc("TRN2", target_bir_lowering=False, debug=False,
                   num_devices=N_CORES)

    xt = nc.declare_dram_parameter("xt", [K, TPC], dt.float32, isOutput=False)
    wt = nc.declare_dram_parameter("wt", [MT, K, P], dt.float32, isOutput=False)
    bias = nc.declare_dram_parameter("bias", [M], dt.float32, isOutput=False)
    out = nc.declare_dram_parameter("out", [M, TPC], dt.float32, isOutput=True)

    with tile.TileContext(nc) as tc:
        with (
            tc.tile_pool(name="xq", bufs=1) as xq_pool,
            tc.tile_pool(name="xs", bufs=3) as xs_pool,
            tc.tile_pool(name="wsf", bufs=4) as wsf_pool,
            tc.tile_pool(name="wsb", bufs=2) as wsb_pool,
            tc.tile_pool(name="wq", bufs=4) as wq_pool,
            tc.tile_pool(name="cst", bufs=1) as cst_pool,
            tc.tile_pool(name="outp", bufs=6) as out_pool,
            tc.tile_pool(name="ps", bufs=4, space="PSUM") as psum_pool,
            tc.tile_pool(name="junk", bufs=1) as junk_pool,
        ):
            bias_sb = cst_pool.tile([P, MT], dt.float32, name="bias_sb")
            nc.sync.dma_start(bias_sb[:], bias[:].rearrange("(o p) -> p o", p=P))

            # PE warmup: junk matmuls burn the pstate ramp while the first
            # x/w chunks quantize; sized to retire just as real work is ready.
            junk_sb = junk_pool.tile([P, TF], dt.float16, name="junk_sb")
            nc.vector.memset(junk_sb[:], 1.0)

            def junk_mms(n, tgt):
                for _ in range(n):
                    nc.tensor.matmul(tgt[:], junk_sb[:, :P], junk_sb[:],
                                     start=True, stop=True)

            # Quantization: dst (fp16) = clip(round(src * scale), -127, 127)
            # + 1536; the fp32->fp16 convert performs the RNE integer round
            # (fp16 spacing is 1.0 on [1024, 2048)), the DVE dual-op clips,
            # a final subtract recenters.  Chains alternate which engine
            # takes the affine/subtract to balance ScalarE vs DVE.
            xq_tiles = [
                xq_pool.tile([P, 2, TPC], dt.float16, name=f"xq{p}",
                             tag=f"xq{p}")
                for p in range(KT // 2)
            ]

            def quant_chain(dst, src, scale, on_scalar):
                # dst fp16 slice; src fp32 staging (same shape)
                if on_scalar:
                    nc.scalar.activation(dst, src, AF.Copy,
                                         bias=MAGIC, scale=scale)
                    nc.vector.tensor_scalar(dst, dst, HI, LO, OP.min, OP.max)
                    nc.vector.tensor_scalar(dst, dst, MAGIC, None,
                                            OP.subtract)
                else:
                    nc.vector.tensor_scalar(dst, src, scale, MAGIC,
                                            OP.mult, OP.add)
                    nc.vector.tensor_scalar(dst, dst, HI, LO, OP.min, OP.max)
                    nc.scalar.activation(dst, dst, AF.Copy, bias=-MAGIC)

            def quant_x_pair(kp, split=False):
                kt0 = 2 * kp
                xs = xs_pool.tile([P, 2, TPC], dt.float32, name="xs")
                dst = xq_tiles[kp]
                if split:
                    # per-k-tile DMAs + chains on opposite engines: fastest
                    # possible first xq k-tile for the prologue
                    for h in range(2):
                        nc.sync.dma_start(
                            xs[:, h, :],
                            xt[(kt0 + h) * P:(kt0 + h + 1) * P, :]
                            .rearrange("(o p) t -> p o t", p=P))
                        quant_chain(dst[:, h, :], xs[:, h, :], s_x,
                                    on_scalar=(h == 0))
                else:
                    nc.sync.dma_start(
                        xs[:],
                        xt[kt0 * P:(kt0 + 2) * P, :]
                        .rearrange("(o p) t -> p o t", p=P))
                    quant_chain(dst[:], xs[:], s_x, on_scalar=(kp % 2 == 0))

            def prep_w_fine(mt, subs):
                # prologue m-tiles: sub-chunk chains write disjoint k-slices
                # of one wq tile so the first matmuls only wait on sub 0.
                wq = wq_pool.tile([P, KT, P], dt.float16, name="wq",
                                  tag="wq")
                for i, (a, b) in enumerate(subs):
                    ws = wsf_pool.tile([P, b - a, P], dt.float32, name="wsf")
                    nc.sync.dma_start(
                        ws[:],
                        wt[mt, a * P:b * P, :]
                        .rearrange("(o p) f -> p o f", p=P))
                    quant_chain(wq[:, a:b, :], ws[:], s_w,
                                on_scalar=(i % 2 == 0))
                return wq

            def prep_w_big(mt):
                # steady state: 4 quarter DMAs (parallel queues), whole-tile
                # quant chain (fewest per-op fixed costs)
                wq = wq_pool.tile([P, KT, P], dt.float16, name="wq", tag="wq")
                ws = wsb_pool.tile([P, KT, P], dt.float32, name="wsb")
                q = KT // 4
                for h in range(4):
                    nc.sync.dma_start(
                        ws[:, h * q:(h + 1) * q, :],
                        wt[mt, h * q * P:(h + 1) * q * P, :]
                        .rearrange("(o p) f -> p o f", p=P))
                quant_chain(wq[:], ws[:], s_w, on_scalar=(mt % 2 == 0))
                return wq

            def alloc_ps():
                return [psum_pool.tile([P, TF], dt.float32, name=f"ps{i}")
                        for i in range(NTF)]

            def mm(pss, wq, kt, start, stop):
                for tf in range(NTF):
                    nc.tensor.matmul(
                        pss[tf][:],
                        wq[:, kt, :],
                        xq_tiles[kt // 2][:, kt % 2,
                                          tf * TF:(tf + 1) * TF],
                        start=start, stop=stop,
                    )

            def store(mt, pss):
                for tf in range(NTF):
                    outt = out_pool.tile([P, TF], dt.float32, name="outt")
                    nc.scalar.activation(
                        outt[:], pss[tf][:],
                        AF.Identity, bias=bias_sb[:, mt:mt + 1], scale=inv_s,
                    )
                    nc.sync.dma_start(
                        out[mt * P:(mt + 1) * P, tf * TF:(tf + 1) * TF],
                        outt[:])

            # Fused prologue: quantize x chunk-by-chunk, staggered with the
            # first PRO m-tiles' (fine-grained) weight prep; after each
            # chunk, run the matmuls that are newly enabled (psum
            # accumulation k-order is free, each m-tile still sees chunks in
            # order).  4 m-tiles x 2 psum banks fill PSUM; the warmup
            # matmuls share the 4th m-tile's bank (its real start=True
            # matmul resets it later, WAW-serialized by Tile).
            PRO = min(4, MT)
            wqs = {}
            pro_ps = {}
            pro_ps[PRO - 1] = alloc_ps()
            junk_mms(JUNK, pro_ps[PRO - 1][0])
            quant_x_pair(0, split=True)
            wqs[0] = prep_w_fine(0, [(0, 4), (4, 8), (8, 16), (16, 24),
                                     (24, 32)])
            quant_x_pair(1)
            pro_ps[0] = alloc_ps()
            PPC = max(KPC // 2, 1)  # pairs per chunk
            for c in range(XCH):
                for k in range(PPC):
                    if c == 0 and k < 2:
                        continue
                    quant_x_pair(c * PPC + k)
                if c + 1 < PRO:
                    wqs[c + 1] = prep_w_fine(
                        c + 1, [(0, 8), (8, 16), (16, 24), (24, 32)])
                    if c + 1 not in pro_ps:
                        pro_ps[c + 1] = alloc_ps()
                for mt in range(PRO):
                    cc = c - mt
                    if 0 <= cc < XCH:
                        for k in range(KPC):
                            kt = cc * KPC + k
                            mm(pro_ps[mt], wqs[mt], kt,
                               start=(kt == 0), stop=(kt == KT - 1))
            for mt in range(PRO):
                for cc in range(XCH - mt, XCH):
                    for k in range(KPC):
                        kt = cc * KPC + k
                        mm(pro_ps[mt], wqs[mt], kt,
                           start=(kt == 0), stop=(kt == KT - 1))
                store(mt, pro_ps[mt])

            # Steady-state m-loop, software-pipelined two m-tiles ahead.
            pending = {}
            for mt in range(PRO, min(PRO + 2, MT)):
                pending[mt] = prep_w_big(mt)
            for mt in range(PRO, MT):
                wq = pending.pop(mt)
                if mt + 2 < MT:
                    pending[mt + 2] = prep_w_big(mt + 2)
                pss = alloc_ps()
                for kt in range(KT):
                    mm(pss, wq, kt, start=(kt == 0), stop=(kt == KT - 1))
                store(mt, pss)

    nc.compile()
    return nc

def _prep(x, weight, bias, amax_x, amax_w):
    ax = np.float32(np.asarray(amax_x, dtype=np.float32).reshape(-1)[0])
    aw = np.float32(np.asarray(amax_w, dtype=np.float32).reshape(-1)[0])
    s_x = np.float32(127.0) / ax
    s_w = np.float32(127.0) / aw
    inv_s = np.float32(1.0) / (s_x * s_w)

    x = np.asarray(x, dtype=np.float32)
    weight = np.asarray(weight, dtype=np.float32)
    bias = np.asarray(bias, dtype=np.float32)

    xT = np.ascontiguousarray(x.T)  # [K, N]
    # [MT, K, 128]: per m-tile a contiguous k-major block of W^T
    wt3 = np.ascontiguousarray(weight.reshape(MT, P, K).transpose(0, 2, 1))
    in_maps = [
        {
            "xt": np.ascontiguousarray(xT[:, c * TPC:(c + 1) * TPC]),
            "wt": wt3,
            "bias": bias,
        }
        for c in range(N_CORES)
    ]
    return float(s_x), float(s_w), float(inv_s), in_maps


def _spot_check(full, x, weight, bias, amax_x, amax_w, n=8):
    """Cheap host-side validation of a few output elements against the exact
    quantized-GEMM reference; catches transient device faults (observed as
    both exec errors and corrupted outputs on this fleet)."""
    rng = np.random.default_rng(0)
    ii = rng.integers(0, x.shape[0], size=n)
    jj = rng.integers(0, weight.shape[0], size=n)
    ax = np.float32(np.asarray(amax_x, np.float32).reshape(-1)[0])
    aw = np.float32(np.asarray(amax_w, np.float32).reshape(-1)[0])
    s_x = np.float32(127.0) / ax
    s_w = np.float32(127.0) / aw
    for i, j in zip(ii, jj):
        qx = np.clip(np.round(x[i].astype(np.float32) * s_x), -127, 127)
        qw = np.clip(np.round(weight[j].astype(np.float32) * s_w), -127, 127)
        exp = float(qx @ qw) / float(s_x * s_w) + float(bias[j])
        if abs(float(full[i, j]) - exp) > 1e-2 * max(1.0, abs(exp)):
            return False
    return True


def run(x, weight, bias, amax_x, amax_w, trace: bool = False):
    from concourse.bass_utils import run_bass_kernel_spmd

    s_x, s_w, inv_s, in_maps = _prep(x, weight, bias, amax_x, amax_w)
    nc = build(s_x, s_w, inv_s)
    full = None
    res = None
    err = None
    for attempt in range(3):
        try:
            res = run_bass_kernel_spmd(nc, in_maps,
                                       core_ids=list(range(N_CORES)),
                                       trace=trace)
            shards = [res.results[c]["out"] for c in range(N_CORES)]
            full = np.concatenate([s.T for s in shards],
                                  axis=0).astype(np.float32)
            if _spot_check(full, x, weight, bias, amax_x, amax_w):
                return full, res
        except Exception as e:  # transient NRT exec faults: retry
            err = e
    if full is not None:
        return full, res
    raise err


def kernel(x, weight, bias, amax_x, amax_w):
    full, _ = run(x, weight, bias, amax_x, amax_w, trace=False)
    return full
